# revision 10
# baseline (speedup 1.0000x reference)
"""Trainium2 Bass kernel for a GPT-2-style transformer block (v2).

B=4, T=1024, C=768, H=12 heads (HD=64). 8 NeuronCores.

Sharding: 2 cores per batch sequence; each core owns 512 query tokens
(block-permuted so its query blocks sit at even local block positions),
computes K/V for the full local sequence, causal attention for its
queries, and the MLP for its tokens.

v2 design vs baseline:
- Everything feature-major on chip: the host ships x^T (and takes out^T
  back), so there are NO PE transposes anywhere.
- All matmuls in bf16 (fp32 matmuls cost 2x stream + 6x LDWEIGHTS).
- All weights/biases are host-prepermuted to [128, o, n] contiguous
  layouts -> large-element DMAs only (no 4-byte gather descriptors).
- LN1/LN2 both via ones-column matmul stats + broadcast matmuls.
- Attention computes only the 20 causally-needed 128x128 block pairs
  (vs 24), with a static tril mask on even-diagonal blocks and a
  per-core 0/1 flag on odd-diagonal blocks.
- FC keeps no rank-1 LN correction matmuls: x-hat is normalized once.
- gelu tail uses sigmoid(2z) == 0.5*(1+tanh(z)).
"""

import numpy as np
import ml_dtypes

P = 128
B, T, C, H = 4, 1024, 768, 12
HD = C // H        # 64
CJ = C // P        # 6 C-chunks
NT = T // P        # 8 token blocks
TQ = 512           # own query tokens per core
NQT = TQ // P      # 4 q slots
FC = 4 * C         # 3072
FCJ = FC // P      # 24
NPAIR = H // 2     # 6 head pairs
# gelu: tanh arg z = sqrt(2/pi)*0.044715*x^4 = GA2*x^4;  u = sigmoid(2*z)
GA2 = 0.035677408136300527
N_CORES = 8

_CACHED = {}


def _build_nc():
    import concourse.bass as bass
    from concourse import bacc, mybir
    import concourse.tile as tile
    from contextlib import ExitStack

    F32 = mybir.dt.float32
    BF16 = mybir.dt.bfloat16
    AF = mybir.ActivationFunctionType
    ALU = mybir.AluOpType

    nc = bacc.Bacc()

    xt_d = nc.declare_dram_parameter("xt", [C, T], BF16, isOutput=False)
    xto_d = nc.declare_dram_parameter("xto", [C, TQ], F32, isOutput=False)
    wv_d = nc.declare_dram_parameter("wv", [P, CJ, C], BF16, isOutput=False)
    wk_d = nc.declare_dram_parameter("wk", [P, CJ, C], BF16, isOutput=False)
    wq_d = nc.declare_dram_parameter("wq", [P, CJ, C], BF16, isOutput=False)
    wo_d = nc.declare_dram_parameter("wo", [P, CJ, C], BF16, isOutput=False)
    wfc_d = nc.declare_dram_parameter("wfc", [P, CJ, FC], BF16, isOutput=False)
    wpj_d = nc.declare_dram_parameter("wpj", [P, FCJ, C], BF16, isOutput=False)
    bkq_d = nc.declare_dram_parameter("bkq", [P, 2 * NPAIR], F32, isOutput=False)
    bv_d = nc.declare_dram_parameter("bv", [C], F32, isOutput=False)
    bob_d = nc.declare_dram_parameter("bob", [P, CJ], F32, isOutput=False)
    bfcb_d = nc.declare_dram_parameter("bfcb", [P, FCJ], F32, isOutput=False)
    bpjb_d = nc.declare_dram_parameter("bpjb", [P, CJ], F32, isOutput=False)
    mask2_d = nc.declare_dram_parameter("mask2", [P, 2 * P], BF16, isOutput=False)
    csfc_d = nc.declare_dram_parameter("csfc", [1, FC], BF16, isOutput=False)
    out_d = nc.declare_dram_parameter("out", [C, TQ], F32, isOutput=True)
    import os
    dbg = os.environ.get("KV2_DEBUG") == "1"
    if dbg:
        dbg_xh = nc.declare_dram_parameter("dbg_xh", [C, T], BF16, isOutput=True)
        dbg_k = nc.declare_dram_parameter("dbg_k", [NPAIR * P, T], BF16, isOutput=True)
        dbg_q = nc.declare_dram_parameter("dbg_q", [NPAIR * P, TQ], BF16, isOutput=True)
        dbg_v = nc.declare_dram_parameter("dbg_v", [NT * P, H * (HD + 1)], BF16, isOutput=True)
        dbg_y = nc.declare_dram_parameter("dbg_y", [NPAIR * P, TQ], BF16, isOutput=True)
        dbg_x1 = nc.declare_dram_parameter("dbg_x1", [C, TQ], F32, isOutput=True)
        dbg_xh2 = nc.declare_dram_parameter("dbg_xh2", [C, TQ], BF16, isOutput=True)
        dbg_h1 = nc.declare_dram_parameter("dbg_h1", [FC, TQ], BF16, isOutput=True)

    def bcast_dma(engine, dst, dram_handle, offset, n):
        """DMA [n] DRAM vector broadcast across 128 partitions -> dst[128, n]."""
        ap = dram_handle[:]
        src = bass.AP(tensor=ap.tensor, offset=offset, ap=[[0, P], [1, n]])
        engine.dma_start(dst, src)

    with tile.TileContext(nc) as tc, ExitStack() as ctx:
        persist = ctx.enter_context(tc.tile_pool(name="persist", bufs=1))
        work = ctx.enter_context(tc.tile_pool(name="work", bufs=4))

        # ---------- constants / small loads ----------
        onesc_bf = persist.tile([P, 1], BF16, tag="onescbf")
        nc.vector.memset(onesc_bf, 1.0)
        ones_row = persist.tile([1, P], BF16, tag="onesrow")
        nc.vector.memset(ones_row, 1.0)
        mones_row = persist.tile([1, P], BF16, tag="monesrow")
        nc.vector.memset(mones_row, -1.0)
        ones_hd = persist.tile([1, HD], BF16, tag="oneshd")
        nc.vector.memset(ones_hd, 1.0)
        eps1 = persist.tile([1, 1], F32, tag="eps1")
        nc.vector.memset(eps1, 1e-5)
        # dummy Ln at t=0 pulls the ln/exp ACT table load (~2.7us) off
        # the LN1 critical path (ACT is otherwise idle at kernel start)
        warm = persist.tile([1, 1], F32, tag="warm")
        nc.scalar.activation(warm, eps1, AF.Ln)

        mask2_m = persist.tile([P, 2, P], BF16, tag="mask2")
        nc.gpsimd.dma_start(mask2_m, mask2_d[:, :].rearrange("p (a b) -> p a b", a=2))
        csfc_t = persist.tile([1, FC], BF16, tag="csfc")
        nc.gpsimd.dma_start(csfc_t, csfc_d[:, :])
        bkq_t = persist.tile([P, 2 * NPAIR], F32, tag="bkq")
        nc.gpsimd.dma_start(bkq_t, bkq_d[:, :])
        bob_t = persist.tile([P, CJ], F32, tag="bob")
        nc.gpsimd.dma_start(bob_t, bob_d[:, :])
        bfcb_t = persist.tile([P, FCJ], F32, tag="bfcb")
        nc.gpsimd.dma_start(bfcb_t, bfcb_d[:, :])
        bpjb_t = persist.tile([P, CJ], F32, tag="bpjb")
        nc.gpsimd.dma_start(bpjb_t, bpjb_d[:, :])
        bv_b = persist.tile([P, C], F32, tag="bvb")
        bcast_dma(nc.gpsimd, bv_b, bv_d, 0, C)

        # All big weight DMAs are issued HERE, at the top of the gpsimd
        # instruction stream, so no compute op can delay the issue.  The
        # gpsimd DMA queue drains them serially (~35us total) long before
        # each consumer phase needs its data.  ctx_ad-scoped tiles are
        # freed after phase D (closed below) so phase F fits in SBUF.
        wpj_p = ctx.enter_context(tc.tile_pool(name="wpj_p", bufs=1))
        wpj_t = wpj_p.tile([P, FCJ, C], BF16, tag="wpj")
        ctx_ad = ExitStack()
        wbig = ctx_ad.enter_context(tc.tile_pool(name="wbig", bufs=1))
        wv_t = wbig.tile([P, CJ, C], BF16, tag="wv")
        nc.gpsimd.dma_start(wv_t, wv_d[:, :, :])
        wk_t = wbig.tile([P, CJ, C], BF16, tag="wk")
        nc.gpsimd.dma_start(wk_t, wk_d[:, :, :])
        wq_t = wbig.tile([P, CJ, C], BF16, tag="wq")
        nc.gpsimd.dma_start(wq_t, wq_d[:, :, :])
        wo_t = wbig.tile([P, CJ, C], BF16, tag="wo")
        nc.gpsimd.dma_start(wo_t, wo_d[:, :, :])
        nc.gpsimd.dma_start(wpj_t, wpj_d[:, :, :])
        # first FC weight piece lives outside the A-D scope so its DMA
        # doesn't wait on freed-pool WAR hazards
        wfc0_t = wpj_p.tile([P, CJ, 4 * P], BF16, tag="wfc0")
        nc.sync.dma_start(wfc0_t, wfc_d[:, :, 0:4 * P])

        # x^T arrives pre-cast to bf16; a separate fp32 copy of the own
        # (query) columns feeds the phase-D residual at full precision.
        xbf = [wbig.tile([P, T], BF16, tag=f"xbf{m}", name=f"xbft{m}")
               for m in range(CJ)]
        for m in range(CJ):
            nc.sync.dma_start(xbf[m], xt_d[m * P:(m + 1) * P, :])
        xTo = [wbig.tile([P, TQ], F32, tag=f"xTo{m}", name=f"xTo{m}")
               for m in range(CJ)]
        for m in range(CJ):
            nc.sync.dma_start(xTo[m], xto_d[m * P:(m + 1) * P, :])
        xh = [wbig.tile([P, T], BF16, tag=f"xh{m}", name=f"xh{m}")
              for m in range(CJ)]
        x1T = [persist.tile([P, TQ], F32, tag=f"x1T{m}", name=f"x1T{m}")
               for m in range(CJ)]

        def own_cols(t):
            """[P, NQT, P] strided view of a [P, T] tile: even 128-blocks."""
            return t.rearrange("p (b c) -> p b c", c=P)[:, 0::2, :]

        # ---------------- phase A: LN1 (feature-major) ----------------
        with tc.tile_pool(name="ph_a", bufs=2) as ph_a, \
             tc.tile_pool(name="lna", bufs=1) as lna, \
             tc.tile_pool(name="ps_st", bufs=1, space="PSUM") as ps_st, \
             tc.tile_pool(name="ps_bc", bufs=1, space="PSUM") as ps_bc:
            mu_ps = ps_st.tile([1, 2, TQ], F32, tag="mups", name="mups")
            sq_ps = ps_st.tile([1, 2, TQ], F32, tag="sqps", name="sqps")
            for m in range(CJ):
                b = xbf[m]
                s = ph_a.tile([P, T], BF16, tag="xsq")
                nc.vector.tensor_tensor(s, b, b, ALU.mult)
                for hf in range(2):
                    nc.tensor.matmul(mu_ps[:, hf, :], onesc_bf,
                                     b[:, hf * TQ:(hf + 1) * TQ],
                                     start=(m == 0), stop=(m == CJ - 1),
                                     skip_group_check=True)
                for hf in range(2):
                    nc.tensor.matmul(sq_ps[:, hf, :], onesc_bf,
                                     s[:, hf * TQ:(hf + 1) * TQ],
                                     start=(m == 0), stop=(m == CJ - 1),
                                     skip_group_check=True)
            mu_f = lna.tile([1, T], F32, tag="muf")
            nc.vector.tensor_scalar(mu_f, mu_ps.rearrange("o a b -> o (a b)"),
                                    1.0 / C, None, ALU.mult)
            var_f = lna.tile([1, T], F32, tag="varf")
            nc.vector.tensor_scalar(var_f, sq_ps.rearrange("o a b -> o (a b)"),
                                    1.0 / C, None, ALU.mult)
            musq = lna.tile([1, T], F32, tag="musq")
            nc.vector.tensor_tensor(musq, mu_f, mu_f, ALU.mult)
            nc.vector.tensor_tensor(var_f, var_f, musq, ALU.subtract)
            # rstd = exp(-0.5*ln(var+eps)): keeps ACT on the ln/exp table
            # set (shared with attention's exp -> no mid-kernel table load)
            lv_f = lna.tile([1, T], F32, tag="lvf")
            nc.scalar.activation(lv_f, var_f, AF.Ln, bias=eps1)
            rstd_f = lna.tile([1, T], F32, tag="rstdf")
            nc.scalar.activation(rstd_f, lv_f, AF.Exp, scale=-0.5)
            nmr_f = lna.tile([1, T], F32, tag="nmrf")
            nc.vector.tensor_tensor(nmr_f, mu_f, rstd_f, ALU.mult)
            rstd_bf = lna.tile([1, T], BF16, tag="rstdbf")
            nc.vector.tensor_copy(rstd_bf, rstd_f)
            nmr_bf = lna.tile([1, T], BF16, tag="nmrbf")
            nc.vector.tensor_copy(nmr_bf, nmr_f)
            # broadcast across partitions via K=1 matmuls
            rst_ps = ps_bc.tile([P, 2, TQ], F32, tag="rstps", name="rstps")
            nrm_ps = ps_bc.tile([P, 2, TQ], F32, tag="nrmps", name="nrmps")
            for hf in range(2):
                nc.tensor.matmul(rst_ps[:, hf, :], ones_row,
                                 rstd_bf[:, hf * TQ:(hf + 1) * TQ],
                                 start=True, stop=True)
                nc.tensor.matmul(nrm_ps[:, hf, :], mones_row,
                                 nmr_bf[:, hf * TQ:(hf + 1) * TQ],
                                 start=True, stop=True)
            rst_s = lna.tile([P, T], F32, tag="rsts")
            nc.vector.tensor_copy(rst_s, rst_ps.rearrange("p a b -> p (a b)"))
            nrm_s = lna.tile([P, T], F32, tag="nrms")
            nc.scalar.copy(nrm_s, nrm_ps.rearrange("p a b -> p (a b)"))
            # x-hat = x*rstd - mu*rstd (bf16 out), token-half-split so the
            # first half's K/V matmuls unblock ~6us sooner
            for th in range(2):
                sl = slice(th * TQ, (th + 1) * TQ)
                for m in range(CJ):
                    tmp = ph_a.tile([P, TQ], F32, tag="xnt")
                    nc.vector.tensor_tensor(tmp, xbf[m][:, sl],
                                            rst_s[:, sl], ALU.mult)
                    nc.vector.tensor_tensor(xh[m][:, sl], tmp,
                                            nrm_s[:, sl], ALU.add)

        if dbg:
            for m in range(CJ):
                nc.sync.dma_start(dbg_xh[m * P:(m + 1) * P, :], xh[m])

        # ---------------- phases B+C: V, K, Q, attention ----------------
        with tc.tile_pool(name="attn_live", bufs=1) as attn_live:
            v_aug = [attn_live.tile([P, H, HD + 1], BF16, tag=f"vaug{t}",
                                    name=f"vaug{t}") for t in range(NT)]
            kTp = [attn_live.tile([P, T], BF16, tag=f"kTp{m}", name=f"kTp{m}")
                   for m in range(NPAIR)]
            qTp = [attn_live.tile([P, TQ], BF16, tag=f"qTp{m}", name=f"qTp{m}")
                   for m in range(NPAIR)]
            yTp = [attn_live.tile([P, TQ], BF16, tag=f"yTp{m}", name=f"yTp{m}")
                   for m in range(NPAIR)]

            # ---- phase B: V (token-major) + ones column ----
            with tc.tile_pool(name="ps_v", bufs=2, space="PSUM") as ps_v:
                for t in range(NT):
                    nc.vector.memset(v_aug[t][:, :, HD:HD + 1], 1.0)
                for t in range(NT):
                    pv = ps_v.tile([P, 2, TQ], F32, tag="pv")
                    for half in range(2):
                        for kc in range(CJ):
                            nc.tensor.matmul(
                                pv[:, half, 0:384],
                                xh[kc][:, t * P:(t + 1) * P],
                                wv_t[:, kc, half * 384:(half + 1) * 384],
                                start=(kc == 0), stop=(kc == CJ - 1))
                    for half in range(2):
                        nc.vector.tensor_tensor(
                            v_aug[t][:, half * 6:(half + 1) * 6, 0:HD],
                            pv[:, half, 0:384].rearrange("p (h d) -> p h d", d=HD),
                            bv_b[:, half * 384:(half + 1) * 384].rearrange(
                                "p (h d) -> p h d", d=HD),
                            ALU.add)

            # ---- phase C: K, Q, attention per head pair ----
            with tc.tile_pool(name="att", bufs=3) as att, \
                 tc.tile_pool(name="ps_kq", bufs=2, space="PSUM") as ps_kq, \
                 tc.tile_pool(name="ps_sc", bufs=2, space="PSUM") as ps_sc, \
                 tc.tile_pool(name="ps_av", bufs=2, space="PSUM") as ps_av:
                for m in range(NPAIR):
                        # K for pair m: full 1024 tokens
                        for tk in range(2):
                            pk = ps_kq.tile([P, TQ], F32, tag="kq")
                            for kc in range(CJ):
                                nc.tensor.matmul(
                                    pk, wk_t[:, kc, m * P:(m + 1) * P],
                                    xh[kc][:, tk * TQ:(tk + 1) * TQ],
                                    start=(kc == 0), stop=(kc == CJ - 1))
                            nc.scalar.activation(
                                kTp[m][:, tk * TQ:(tk + 1) * TQ], pk,
                                AF.Identity, bias=bkq_t[:, m:m + 1])
                        # Q for pair m: own 512 tokens
                        pq = ps_kq.tile([P, TQ], F32, tag="kq")
                        for kc in range(CJ):
                            nc.tensor.matmul(
                                pq, wq_t[:, kc, m * P:(m + 1) * P],
                                own_cols(xh[kc]),
                                start=(kc == 0), stop=(kc == CJ - 1))
                        nc.scalar.activation(
                            qTp[m], pq, AF.Identity,
                            bias=bkq_t[:, NPAIR + m:NPAIR + m + 1])

                        # attention for heads (2m, 2m+1), interleaved
                        avs = [ps_av.tile([HD + 1, TQ], F32, tag="av",
                                          name=f"av{m}_{hh}")
                               for hh in range(2)]
                        for j in range(NQT):
                            w = TQ - j * P
                            q0 = j * P
                            for hh in range(2):
                                h = 2 * m + hh
                                hs = slice(hh * HD, (hh + 1) * HD)
                                sc = ps_sc.tile([P, 2, TQ], F32, tag="sc")
                                for i in range(2):
                                    nc.tensor.matmul(
                                        sc[:, i, 0:w],
                                        kTp[m][hs, (2 * j + i) * P:
                                               (2 * j + i + 1) * P],
                                        qTp[m][hs, q0:TQ],
                                        start=True, stop=True)
                                ex = att.tile([P, 2, TQ], BF16, tag="ex")
                                nc.scalar.activation(
                                    ex[:, :, 0:w], sc[:, :, 0:w],
                                    AF.Exp, scale=0.125)
                                # diagonal masks: even -> tril, odd -> flag
                                nc.vector.tensor_tensor(
                                    ex[:, :, 0:P], ex[:, :, 0:P],
                                    mask2_m, ALU.mult)
                                for i in range(2):
                                    nc.tensor.matmul(
                                        avs[hh][:, q0:TQ],
                                        v_aug[2 * j + i][:, h, :],
                                        ex[:, i, 0:w],
                                        start=(j == 0 and i == 0),
                                        stop=(j == NQT - 1 and i == 1),
                                        skip_group_check=True)
                        for hh in range(2):
                            av = avs[hh]
                            sums_bf = att.tile([1, TQ], BF16, tag="sums")
                            nc.vector.tensor_copy(sums_bf, av[HD:HD + 1, :])
                            dn = ps_sc.tile([HD, TQ], F32, tag="sc")
                            nc.tensor.matmul(dn, ones_hd, sums_bf,
                                             start=True, stop=True)
                            rb = att.tile([HD, TQ], F32, tag="rb")
                            with nc.allow_low_precision(reason="softmax denom"):
                                nc.vector.reciprocal_approx_fast(rb, dn)
                            nc.vector.tensor_tensor(
                                yTp[m][hh * HD:(hh + 1) * HD, :],
                                av[0:HD, :], rb, ALU.mult)

            if dbg:
                for m in range(NPAIR):
                    nc.sync.dma_start(dbg_k[m * P:(m + 1) * P, :], kTp[m])
                    nc.sync.dma_start(dbg_q[m * P:(m + 1) * P, :], qTp[m])
                    nc.sync.dma_start(dbg_y[m * P:(m + 1) * P, :], yTp[m])
                for t in range(NT):
                    nc.sync.dma_start(
                        dbg_v[t * P:(t + 1) * P, :],
                        v_aug[t].rearrange("p h d -> p (h d)"))

            # ---- phase D: Wo + bias + residual -> x1T ----
            with tc.tile_pool(name="ps_wo", bufs=2, space="PSUM") as ps_wo:
                for m in range(CJ):
                    pm = ps_wo.tile([P, TQ], F32, tag="wops")
                    for kc in range(CJ):
                        nc.tensor.matmul(
                            pm, wo_t[:, kc, m * P:(m + 1) * P], yTp[kc],
                            start=(kc == 0), stop=(kc == CJ - 1))
                    nc.vector.tensor_scalar(
                        x1T[m], pm, bob_t[:, m:m + 1], None, ALU.add)
                    nc.vector.tensor_tensor(
                        x1T[m], x1T[m], xTo[m], ALU.add)

        if dbg:
            for m in range(CJ):
                nc.sync.dma_start(dbg_x1[m * P:(m + 1) * P, :], x1T[m])

        ctx_ad.close()

        # ---------------- phases E-G: LN2, FC+gelu, proj ----------------
        with tc.tile_pool(name="mlp_live", bufs=1) as mlp_live:
            h1T = [mlp_live.tile([P, TQ], BF16, tag=f"h1T{m}", name=f"h1T{m}")
                   for m in range(FCJ)]
            xh2 = [mlp_live.tile([P, TQ], BF16, tag=f"xh2{m}", name=f"xh2{m}")
                   for m in range(CJ)]

            # ---- phase E: LN2 stats only (rank-1 folded into FC) ----
            nmu2_bf = mlp_live.tile([1, TQ], BF16, tag="nmu2bf")
            rst2_s = mlp_live.tile([P, TQ], F32, tag="rst2s")
            with tc.tile_pool(name="ph_e", bufs=2) as ph_e, \
                 tc.tile_pool(name="ps_st2", bufs=1, space="PSUM") as ps_st2, \
                 tc.tile_pool(name="ps_bc2", bufs=1, space="PSUM") as ps_bc2:
                mu_ps = ps_st2.tile([1, TQ], F32, tag="mups2", name="mups2")
                sq_ps = ps_st2.tile([1, TQ], F32, tag="sqps2", name="sqps2")
                for m in range(CJ):
                    b = xh2[m]
                    s = ph_e.tile([P, TQ], BF16, tag="x1sq")
                    nc.vector.tensor_copy(b, x1T[m])
                    nc.vector.tensor_tensor(s, b, b, ALU.mult)
                    nc.tensor.matmul(mu_ps, onesc_bf, b,
                                     start=(m == 0), stop=(m == CJ - 1),
                                     skip_group_check=True)
                    nc.tensor.matmul(sq_ps, onesc_bf, s,
                                     start=(m == 0), stop=(m == CJ - 1),
                                     skip_group_check=True)
                nmu_f = ph_e.tile([1, TQ], F32, tag="nmuf2")
                nc.vector.tensor_scalar(nmu_f, mu_ps, -1.0 / C, None, ALU.mult)
                var_f = ph_e.tile([1, TQ], F32, tag="varf2")
                nc.vector.tensor_scalar(var_f, sq_ps, 1.0 / C, None, ALU.mult)
                musq = ph_e.tile([1, TQ], F32, tag="musq2")
                nc.vector.tensor_tensor(musq, nmu_f, nmu_f, ALU.mult)
                nc.vector.tensor_tensor(var_f, var_f, musq, ALU.subtract)
                lv_f = ph_e.tile([1, TQ], F32, tag="lvf2")
                nc.scalar.activation(lv_f, var_f, AF.Ln, bias=eps1)
                rstd_f = ph_e.tile([1, TQ], F32, tag="rstdf2")
                nc.scalar.activation(rstd_f, lv_f, AF.Exp, scale=-0.5)
                rstd_bf = ph_e.tile([1, TQ], BF16, tag="rstdbf2")
                nc.vector.tensor_copy(rstd_bf, rstd_f)
                nc.vector.tensor_copy(nmu2_bf, nmu_f)
                rst_ps = ps_bc2.tile([P, TQ], F32, tag="rstps2", name="rstps2")
                nc.tensor.matmul(rst_ps, ones_row, rstd_bf,
                                 start=True, stop=True)
                nc.vector.tensor_copy(rst2_s, rst_ps)

            # ---- phase F: FC + gelu ----
            with tc.tile_pool(name="wfc_p", bufs=3) as wfc_p, \
                 tc.tile_pool(name="ph_f", bufs=3) as ph_f, \
                 tc.tile_pool(name="ps_fc", bufs=4, space="PSUM") as ps_fc:
                for mo in range(FCJ):
                    if mo % 4 == 0:
                        if mo == 0:
                            wt4 = wfc0_t
                        else:
                            wt4 = wfc_p.tile([P, CJ, 4 * P], BF16, tag="wfc")
                            nc.sync.dma_start(
                                wt4, wfc_d[:, :, mo * P:(mo + 4) * P])
                    ml = mo % 4
                    pf = ps_fc.tile([P, TQ], F32, tag="fc")
                    nc.tensor.matmul(pf, csfc_t[:, mo * P:(mo + 1) * P],
                                     nmu2_bf,
                                     start=True, stop=False,
                                     skip_group_check=True)
                    for kc in range(CJ):
                        nc.tensor.matmul(pf, wt4[:, kc, ml * P:(ml + 1) * P],
                                         xh2[kc],
                                         start=False, stop=(kc == CJ - 1),
                                         skip_group_check=True)
                    xb = ph_f.tile([P, TQ], BF16, tag="xb")
                    nc.vector.tensor_tensor(xb, pf, rst2_s, ALU.mult)
                    s1 = ph_f.tile([P, TQ], BF16, tag="s1")
                    s2 = ph_f.tile([P, TQ], BF16, tag="s2")
                    nc.vector.tensor_tensor(s1, xb, xb, ALU.mult)
                    nc.vector.tensor_tensor(s2, s1, s1, ALU.mult)
                    u = ph_f.tile([P, TQ], BF16, tag="u")
                    nc.scalar.activation(u, s2, AF.Sigmoid, scale=2.0 * GA2)
                    nc.vector.tensor_tensor(h1T[mo], xb, u, ALU.mult)

            if dbg:
                for m in range(CJ):
                    nc.sync.dma_start(dbg_xh2[m * P:(m + 1) * P, :], xh2[m])
                for m in range(FCJ):
                    nc.sync.dma_start(dbg_h1[m * P:(m + 1) * P, :], h1T[m])

            # ---- phase G: proj + bias + residual -> out (m-outer) ----
            with tc.tile_pool(name="ph_g", bufs=2) as ph_g, \
                 tc.tile_pool(name="ps_pj", bufs=2, space="PSUM") as ps_pj:
                for m in range(CJ):
                    pg = ps_pj.tile([P, TQ], F32, tag="pj")
                    for kc in range(FCJ):
                        nc.tensor.matmul(
                            pg, wpj_t[:, kc, m * P:(m + 1) * P], h1T[kc],
                            start=(kc == 0), stop=(kc == FCJ - 1))
                    oj = ph_g.tile([P, TQ], F32, tag="oj")
                    nc.vector.tensor_scalar(
                        oj, pg, bpjb_t[:, m:m + 1], None, ALU.add)
                    nc.vector.tensor_tensor(oj, oj, x1T[m], ALU.add)
                    nc.sync.dma_start(out_d[m * P:(m + 1) * P, :], oj)

    nc.compile()
    return nc


def _get_nc():
    if "nc" not in _CACHED:
        _CACHED["nc"] = _build_nc()
    return _CACHED["nc"]


def _perm_blocks(p):
    return [p, 1 - p, 2 + p, 3 - p, 4 + p, 5 - p, 6 + p, 7 - p]


def _build_in_maps(x, ln1_scale, ln1_bias, Wqkv, bqkv, Wo, bo,
                   ln2_scale, ln2_bias, Wfc, bfc, Wproj, bproj):
    bf16 = ml_dtypes.bfloat16
    x = np.asarray(x, np.float32)
    # Fold LN scale/bias into the following projection (exact):
    #   ln(x)*s + b  @ W  ==  ln(x) @ (s[:,None]*W)  +  b @ W
    Wq64 = np.asarray(ln1_scale, np.float64)[:, None] * np.asarray(Wqkv, np.float64)
    bq64 = np.asarray(bqkv, np.float64) + np.asarray(ln1_bias, np.float64) @ Wq64
    Wfc64 = np.asarray(ln2_scale, np.float64)[:, None] * np.asarray(Wfc, np.float64)
    bfc64 = np.asarray(bfc, np.float64) + np.asarray(ln2_bias, np.float64) @ Wfc64
    # kernel folds LN2 into FC via a rank-1 matmul and drops the fc bias
    # add (spec fill is zeros); verify that
    assert np.abs(bfc64).max() == 0.0, "nonzero fc bias unsupported"
    # reference packs qkv per head: [h0: q|k|v, h1: q|k|v, ...] -> head-major Q,K,V
    colmap = np.arange(3 * C).reshape(H, 3, HD)
    Wq64 = Wq64.astype(np.float32)
    bq64 = bq64.astype(np.float32)

    def pcm(w):  # [C, n] -> [128, CJ, n] (contraction chunk-major)
        n = w.shape[1]
        return np.ascontiguousarray(
            w.reshape(CJ, P, n).transpose(1, 0, 2).astype(bf16))

    wq_h = Wq64[:, colmap[:, 0, :].ravel()]
    wk_h = Wq64[:, colmap[:, 1, :].ravel()]
    wv_h = Wq64[:, colmap[:, 2, :].ravel()]
    bq_h = bq64[colmap[:, 0, :].ravel()]
    bk_h = bq64[colmap[:, 1, :].ravel()]
    bv_h = bq64[colmap[:, 2, :].ravel()]
    bkq = np.concatenate([bk_h.reshape(NPAIR, P).T,
                          bq_h.reshape(NPAIR, P).T], axis=1)

    shared = {
        "wv": pcm(wv_h), "wk": pcm(wk_h), "wq": pcm(wq_h),
        "wo": pcm(np.asarray(Wo, np.float32)),
        "wfc": pcm(Wfc64.astype(np.float32)),
        "csfc": np.ascontiguousarray(
            Wfc64.astype(np.float32).sum(axis=0)[None, :].astype(bf16)),
        "wpj": np.ascontiguousarray(
            np.asarray(Wproj, np.float32).reshape(FCJ, P, C)
            .transpose(1, 0, 2).astype(bf16)),
        "bkq": np.ascontiguousarray(bkq, dtype=np.float32),
        "bv": np.ascontiguousarray(bv_h, dtype=np.float32),
        "bob": np.ascontiguousarray(
            np.asarray(bo, np.float32).reshape(CJ, P).T),
        "bfcb": np.ascontiguousarray(bfc64.astype(np.float32).reshape(FCJ, P).T),
        "bpjb": np.ascontiguousarray(
            np.asarray(bproj, np.float32).reshape(CJ, P).T),
    }
    tril = (np.arange(P)[None, :] >= np.arange(P)[:, None]).astype(np.float32)
    in_maps = []
    own_toks = []
    for c in range(N_CORES):
        s, p = divmod(c, 2)
        blocks = _perm_blocks(p)
        tok = np.concatenate([np.arange(b * P, (b + 1) * P) for b in blocks])
        own = np.concatenate([np.arange(b * P, (b + 1) * P)
                              for b in blocks[0::2]])
        own_toks.append((s, own))
        mask2 = np.concatenate(
            [tril, np.full((P, P), float(p), np.float32)], axis=1)
        xtl = np.ascontiguousarray(x[s][tok].T)
        in_maps.append({
            "xt": np.ascontiguousarray(xtl.astype(bf16)),
            "xto": np.ascontiguousarray(x[s][own].T),
            "mask2": np.ascontiguousarray(mask2.astype(bf16)),
            **shared,
        })
    return in_maps, own_toks


def kernel(x, ln1_scale, ln1_bias, Wqkv, bqkv, Wo, bo,
           ln2_scale, ln2_bias, Wfc, bfc, Wproj, bproj):
    from concourse.bass_utils import run_bass_kernel_spmd

    in_maps, own_toks = _build_in_maps(
        x, ln1_scale, ln1_bias, Wqkv, bqkv, Wo, bo,
        ln2_scale, ln2_bias, Wfc, bfc, Wproj, bproj)
    nc = _get_nc()
    res = run_bass_kernel_spmd(nc, in_maps, list(range(N_CORES)))

    out = np.empty((B, T, C), np.float32)
    for c in range(N_CORES):
        s, own = own_toks[c]
        out[s][own] = res.results[c]["out"].T
    return out


# revision 13
# speedup vs baseline: 1.0240x; 1.0240x over previous
"""Trainium2 Bass kernel for a GPT-2-style transformer block (v2).

B=4, T=1024, C=768, H=12 heads (HD=64). 8 NeuronCores.

Sharding: 2 cores per batch sequence; each core owns 512 query tokens
(block-permuted so its query blocks sit at even local block positions),
computes K/V for the full local sequence, causal attention for its
queries, and the MLP for its tokens.

v2 design vs baseline:
- Everything feature-major on chip: the host ships x^T (and takes out^T
  back), so there are NO PE transposes anywhere.
- All matmuls in bf16 (fp32 matmuls cost 2x stream + 6x LDWEIGHTS).
- All weights/biases are host-prepermuted to [128, o, n] contiguous
  layouts -> large-element DMAs only (no 4-byte gather descriptors).
- LN1/LN2 both via ones-column matmul stats + broadcast matmuls.
- Attention computes only the 20 causally-needed 128x128 block pairs
  (vs 24), with a static tril mask on even-diagonal blocks and a
  per-core 0/1 flag on odd-diagonal blocks.
- FC keeps no rank-1 LN correction matmuls: x-hat is normalized once.
- gelu tail uses sigmoid(2z) == 0.5*(1+tanh(z)).
"""

import numpy as np
import ml_dtypes

P = 128
B, T, C, H = 4, 1024, 768, 12
HD = C // H        # 64
CJ = C // P        # 6 C-chunks
NT = T // P        # 8 token blocks
TQ = 512           # own query tokens per core
NQT = TQ // P      # 4 q slots
FC = 4 * C         # 3072
FCJ = FC // P      # 24
NPAIR = H // 2     # 6 head pairs
# gelu: tanh arg z = sqrt(2/pi)*0.044715*x^4 = GA2*x^4;  u = sigmoid(2*z)
GA2 = 0.035677408136300527
N_CORES = 8

_CACHED = {}


def _build_nc():
    import concourse.bass as bass
    from concourse import bacc, mybir
    import concourse.tile as tile
    from contextlib import ExitStack

    F32 = mybir.dt.float32
    BF16 = mybir.dt.bfloat16
    AF = mybir.ActivationFunctionType
    ALU = mybir.AluOpType

    nc = bacc.Bacc()

    xt_d = nc.declare_dram_parameter("xt", [C, T], BF16, isOutput=False)
    xto_d = nc.declare_dram_parameter("xto", [C, TQ], F32, isOutput=False)
    wv_d = nc.declare_dram_parameter("wv", [P, CJ, C], BF16, isOutput=False)
    wk_d = nc.declare_dram_parameter("wk", [P, CJ, C], BF16, isOutput=False)
    wq_d = nc.declare_dram_parameter("wq", [P, CJ, C], BF16, isOutput=False)
    wo_d = nc.declare_dram_parameter("wo", [P, CJ, C], BF16, isOutput=False)
    wfc_d = nc.declare_dram_parameter("wfc", [P, CJ, FC], BF16, isOutput=False)
    wpj_d = nc.declare_dram_parameter("wpj", [P, FCJ, C], BF16, isOutput=False)
    bkq_d = nc.declare_dram_parameter("bkq", [P, 2 * NPAIR], F32, isOutput=False)
    bv_d = nc.declare_dram_parameter("bv", [C], F32, isOutput=False)
    bob_d = nc.declare_dram_parameter("bob", [P, CJ], F32, isOutput=False)
    bfcb_d = nc.declare_dram_parameter("bfcb", [P, FCJ], F32, isOutput=False)
    bpjb_d = nc.declare_dram_parameter("bpjb", [P, CJ], F32, isOutput=False)
    mask2_d = nc.declare_dram_parameter("mask2", [P, 2 * P], BF16, isOutput=False)
    out_d = nc.declare_dram_parameter("out", [C, TQ], F32, isOutput=True)
    import os
    dbg = os.environ.get("KV2_DEBUG") == "1"
    if dbg:
        dbg_xh = nc.declare_dram_parameter("dbg_xh", [C, T], BF16, isOutput=True)
        dbg_k = nc.declare_dram_parameter("dbg_k", [NPAIR * P, T], BF16, isOutput=True)
        dbg_q = nc.declare_dram_parameter("dbg_q", [NPAIR * P, TQ], BF16, isOutput=True)
        dbg_v = nc.declare_dram_parameter("dbg_v", [NT * P, H * (HD + 1)], BF16, isOutput=True)
        dbg_y = nc.declare_dram_parameter("dbg_y", [NPAIR * P, TQ], BF16, isOutput=True)
        dbg_x1 = nc.declare_dram_parameter("dbg_x1", [C, TQ], F32, isOutput=True)
        dbg_xh2 = nc.declare_dram_parameter("dbg_xh2", [C, TQ], BF16, isOutput=True)
        dbg_h1 = nc.declare_dram_parameter("dbg_h1", [FC, TQ], BF16, isOutput=True)

    def bcast_dma(engine, dst, dram_handle, offset, n):
        """DMA [n] DRAM vector broadcast across 128 partitions -> dst[128, n]."""
        ap = dram_handle[:]
        src = bass.AP(tensor=ap.tensor, offset=offset, ap=[[0, P], [1, n]])
        engine.dma_start(dst, src)

    with tile.TileContext(nc) as tc, ExitStack() as ctx:
        persist = ctx.enter_context(tc.tile_pool(name="persist", bufs=1))
        work = ctx.enter_context(tc.tile_pool(name="work", bufs=4))

        # ---------- constants / small loads ----------
        onesc_bf = persist.tile([P, 1], BF16, tag="onescbf")
        nc.vector.memset(onesc_bf, 1.0)
        ones_row = persist.tile([1, P], BF16, tag="onesrow")
        nc.vector.memset(ones_row, 1.0)
        mones_row = persist.tile([1, P], BF16, tag="monesrow")
        nc.vector.memset(mones_row, -1.0)
        ones_hd = persist.tile([1, HD], BF16, tag="oneshd")
        nc.vector.memset(ones_hd, 1.0)
        eps1 = persist.tile([1, 1], F32, tag="eps1")
        nc.vector.memset(eps1, 1e-5)
        # dummy Ln at t=0 pulls the ln/exp ACT table load (~2.7us) off
        # the LN1 critical path (ACT is otherwise idle at kernel start)
        warm = persist.tile([1, 1], F32, tag="warm")
        nc.scalar.activation(warm, eps1, AF.Ln)

        mask2_m = persist.tile([P, 2, P], BF16, tag="mask2")
        nc.gpsimd.dma_start(mask2_m, mask2_d[:, :].rearrange("p (a b) -> p a b", a=2))
        bkq_t = persist.tile([P, 2 * NPAIR], F32, tag="bkq")
        nc.gpsimd.dma_start(bkq_t, bkq_d[:, :])
        bob_t = persist.tile([P, CJ], F32, tag="bob")
        nc.gpsimd.dma_start(bob_t, bob_d[:, :])
        bfcb_t = persist.tile([P, FCJ], F32, tag="bfcb")
        nc.gpsimd.dma_start(bfcb_t, bfcb_d[:, :])
        bpjb_t = persist.tile([P, CJ], F32, tag="bpjb")
        nc.gpsimd.dma_start(bpjb_t, bpjb_d[:, :])
        bv_b = persist.tile([P, C], F32, tag="bvb")
        bcast_dma(nc.gpsimd, bv_b, bv_d, 0, C)

        # All big weight DMAs are issued HERE, at the top of the gpsimd
        # instruction stream, so no compute op can delay the issue.  The
        # gpsimd DMA queue drains them serially (~35us total) long before
        # each consumer phase needs its data.  ctx_ad-scoped tiles are
        # freed after phase D (closed below) so phase F fits in SBUF.
        wpj_p = ctx.enter_context(tc.tile_pool(name="wpj_p", bufs=1))
        wpj_t = wpj_p.tile([P, FCJ, C], BF16, tag="wpj")
        ctx_ad = ExitStack()
        wbig = ctx_ad.enter_context(tc.tile_pool(name="wbig", bufs=1))
        wv_t = wbig.tile([P, CJ, C], BF16, tag="wv")
        nc.gpsimd.dma_start(wv_t, wv_d[:, :, :])
        wk_t = wbig.tile([P, CJ, C], BF16, tag="wk")
        nc.gpsimd.dma_start(wk_t, wk_d[:, :, :])
        wq_t = wbig.tile([P, CJ, C], BF16, tag="wq")
        nc.gpsimd.dma_start(wq_t, wq_d[:, :, :])
        wo_t = wbig.tile([P, CJ, C], BF16, tag="wo")
        nc.gpsimd.dma_start(wo_t, wo_d[:, :, :])
        nc.gpsimd.dma_start(wpj_t, wpj_d[:, :, :])
        # first FC weight piece lives outside the A-D scope so its DMA
        # doesn't wait on freed-pool WAR hazards
        wfc0_t = wpj_p.tile([P, CJ, 4 * P], BF16, tag="wfc0")
        nc.sync.dma_start(wfc0_t, wfc_d[:, :, 0:4 * P])

        # x^T arrives pre-cast to bf16; a separate fp32 copy of the own
        # (query) columns feeds the phase-D residual at full precision.
        xbf = [wbig.tile([P, T], BF16, tag=f"xbf{m}", name=f"xbft{m}")
               for m in range(CJ)]
        for m in range(CJ):
            nc.sync.dma_start(xbf[m], xt_d[m * P:(m + 1) * P, :])
        xTo = [wbig.tile([P, TQ], F32, tag=f"xTo{m}", name=f"xTo{m}")
               for m in range(CJ)]
        for m in range(CJ):
            nc.sync.dma_start(xTo[m], xto_d[m * P:(m + 1) * P, :])
        xh = [wbig.tile([P, T], BF16, tag=f"xh{m}", name=f"xh{m}")
              for m in range(CJ)]
        x1T = [persist.tile([P, TQ], F32, tag=f"x1T{m}", name=f"x1T{m}")
               for m in range(CJ)]

        def own_cols(t):
            """[P, NQT, P] strided view of a [P, T] tile: even 128-blocks."""
            return t.rearrange("p (b c) -> p b c", c=P)[:, 0::2, :]

        # ---------------- phase A: LN1 (feature-major) ----------------
        with tc.tile_pool(name="ph_a", bufs=2) as ph_a, \
             tc.tile_pool(name="lna", bufs=1) as lna, \
             tc.tile_pool(name="ps_st", bufs=1, space="PSUM") as ps_st, \
             tc.tile_pool(name="ps_bc", bufs=1, space="PSUM") as ps_bc:
            mu_ps = ps_st.tile([1, 2, TQ], F32, tag="mups", name="mups")
            sq_ps = ps_st.tile([1, 2, TQ], F32, tag="sqps", name="sqps")
            for m in range(CJ):
                b = xbf[m]
                s = ph_a.tile([P, T], BF16, tag="xsq")
                nc.vector.tensor_tensor(s, b, b, ALU.mult)
                for hf in range(2):
                    nc.tensor.matmul(mu_ps[:, hf, :], onesc_bf,
                                     b[:, hf * TQ:(hf + 1) * TQ],
                                     start=(m == 0), stop=(m == CJ - 1),
                                     skip_group_check=True)
                for hf in range(2):
                    nc.tensor.matmul(sq_ps[:, hf, :], onesc_bf,
                                     s[:, hf * TQ:(hf + 1) * TQ],
                                     start=(m == 0), stop=(m == CJ - 1),
                                     skip_group_check=True)
            mu_f = lna.tile([1, T], F32, tag="muf")
            nc.vector.tensor_scalar(mu_f, mu_ps.rearrange("o a b -> o (a b)"),
                                    1.0 / C, None, ALU.mult)
            var_f = lna.tile([1, T], F32, tag="varf")
            nc.vector.tensor_scalar(var_f, sq_ps.rearrange("o a b -> o (a b)"),
                                    1.0 / C, None, ALU.mult)
            musq = lna.tile([1, T], F32, tag="musq")
            nc.vector.tensor_tensor(musq, mu_f, mu_f, ALU.mult)
            nc.vector.tensor_tensor(var_f, var_f, musq, ALU.subtract)
            # rstd = exp(-0.5*ln(var+eps)): keeps ACT on the ln/exp table
            # set (shared with attention's exp -> no mid-kernel table load)
            lv_f = lna.tile([1, T], F32, tag="lvf")
            nc.scalar.activation(lv_f, var_f, AF.Ln, bias=eps1)
            rstd_f = lna.tile([1, T], F32, tag="rstdf")
            nc.scalar.activation(rstd_f, lv_f, AF.Exp, scale=-0.5)
            nmr_f = lna.tile([1, T], F32, tag="nmrf")
            nc.vector.tensor_tensor(nmr_f, mu_f, rstd_f, ALU.mult)
            rstd_bf = lna.tile([1, T], BF16, tag="rstdbf")
            nc.vector.tensor_copy(rstd_bf, rstd_f)
            nmr_bf = lna.tile([1, T], BF16, tag="nmrbf")
            nc.vector.tensor_copy(nmr_bf, nmr_f)
            # broadcast across partitions via K=1 matmuls
            rst_ps = ps_bc.tile([P, 2, TQ], F32, tag="rstps", name="rstps")
            nrm_ps = ps_bc.tile([P, 2, TQ], F32, tag="nrmps", name="nrmps")
            for hf in range(2):
                nc.tensor.matmul(rst_ps[:, hf, :], ones_row,
                                 rstd_bf[:, hf * TQ:(hf + 1) * TQ],
                                 start=True, stop=True)
                nc.tensor.matmul(nrm_ps[:, hf, :], mones_row,
                                 nmr_bf[:, hf * TQ:(hf + 1) * TQ],
                                 start=True, stop=True)
            rst_s = lna.tile([P, T], F32, tag="rsts")
            nc.vector.tensor_copy(rst_s, rst_ps.rearrange("p a b -> p (a b)"))
            nrm_s = lna.tile([P, T], F32, tag="nrms")
            nc.scalar.copy(nrm_s, nrm_ps.rearrange("p a b -> p (a b)"))
            # x-hat = x*rstd - mu*rstd (bf16 out), token-half-split so the
            # first half's K/V matmuls unblock ~6us sooner
            for th in range(2):
                sl = slice(th * TQ, (th + 1) * TQ)
                for m in range(CJ):
                    tmp = ph_a.tile([P, TQ], F32, tag="xnt")
                    nc.vector.tensor_tensor(tmp, xbf[m][:, sl],
                                            rst_s[:, sl], ALU.mult)
                    nc.vector.tensor_tensor(xh[m][:, sl], tmp,
                                            nrm_s[:, sl], ALU.add)

        if dbg:
            for m in range(CJ):
                nc.sync.dma_start(dbg_xh[m * P:(m + 1) * P, :], xh[m])

        # ---------------- phases B+C: V, K, Q, attention ----------------
        with tc.tile_pool(name="attn_live", bufs=1) as attn_live:
            v_aug = [attn_live.tile([P, H, HD + 1], BF16, tag=f"vaug{t}",
                                    name=f"vaug{t}") for t in range(NT)]
            kTp = [attn_live.tile([P, T], BF16, tag=f"kTp{m}", name=f"kTp{m}")
                   for m in range(NPAIR)]
            qTp = [attn_live.tile([P, TQ], BF16, tag=f"qTp{m}", name=f"qTp{m}")
                   for m in range(NPAIR)]
            yTp = [attn_live.tile([P, TQ], BF16, tag=f"yTp{m}", name=f"yTp{m}")
                   for m in range(NPAIR)]

            # ---- phase B: V (token-major) + ones column ----
            with tc.tile_pool(name="ps_v", bufs=2, space="PSUM") as ps_v:
                for t in range(NT):
                    nc.vector.memset(v_aug[t][:, :, HD:HD + 1], 1.0)
                for t in range(NT):
                    pv = ps_v.tile([P, 2, TQ], F32, tag="pv")
                    for half in range(2):
                        for kc in range(CJ):
                            nc.tensor.matmul(
                                pv[:, half, 0:384],
                                xh[kc][:, t * P:(t + 1) * P],
                                wv_t[:, kc, half * 384:(half + 1) * 384],
                                start=(kc == 0), stop=(kc == CJ - 1))
                    for half in range(2):
                        nc.vector.tensor_tensor(
                            v_aug[t][:, half * 6:(half + 1) * 6, 0:HD],
                            pv[:, half, 0:384].rearrange("p (h d) -> p h d", d=HD),
                            bv_b[:, half * 384:(half + 1) * 384].rearrange(
                                "p (h d) -> p h d", d=HD),
                            ALU.add)

            # ---- phase C: K, Q, attention per head pair ----
            with tc.tile_pool(name="att", bufs=3) as att, \
                 tc.tile_pool(name="ps_kq", bufs=2, space="PSUM") as ps_kq, \
                 tc.tile_pool(name="ps_sc", bufs=2, space="PSUM") as ps_sc, \
                 tc.tile_pool(name="ps_av", bufs=2, space="PSUM") as ps_av:
                for m in range(NPAIR):
                        # K for pair m: full 1024 tokens
                        for tk in range(2):
                            pk = ps_kq.tile([P, TQ], F32, tag="kq")
                            for kc in range(CJ):
                                nc.tensor.matmul(
                                    pk, wk_t[:, kc, m * P:(m + 1) * P],
                                    xh[kc][:, tk * TQ:(tk + 1) * TQ],
                                    start=(kc == 0), stop=(kc == CJ - 1))
                            nc.scalar.activation(
                                kTp[m][:, tk * TQ:(tk + 1) * TQ], pk,
                                AF.Identity, bias=bkq_t[:, m:m + 1])
                        # Q for pair m: own 512 tokens
                        pq = ps_kq.tile([P, TQ], F32, tag="kq")
                        for kc in range(CJ):
                            nc.tensor.matmul(
                                pq, wq_t[:, kc, m * P:(m + 1) * P],
                                own_cols(xh[kc]),
                                start=(kc == 0), stop=(kc == CJ - 1))
                        nc.scalar.activation(
                            qTp[m], pq, AF.Identity,
                            bias=bkq_t[:, NPAIR + m:NPAIR + m + 1])

                        # attention for heads (2m, 2m+1), interleaved
                        avs = [ps_av.tile([HD + 1, TQ], F32, tag="av",
                                          name=f"av{m}_{hh}")
                               for hh in range(2)]
                        for j in range(NQT):
                            w = TQ - j * P
                            q0 = j * P
                            for hh in range(2):
                                h = 2 * m + hh
                                hs = slice(hh * HD, (hh + 1) * HD)
                                sc = ps_sc.tile([P, 2, TQ], F32, tag="sc")
                                for i in range(2):
                                    nc.tensor.matmul(
                                        sc[:, i, 0:w],
                                        kTp[m][hs, (2 * j + i) * P:
                                               (2 * j + i + 1) * P],
                                        qTp[m][hs, q0:TQ],
                                        start=True, stop=True)
                                ex = att.tile([P, 2, TQ], BF16, tag="ex")
                                nc.scalar.activation(
                                    ex[:, :, 0:w], sc[:, :, 0:w],
                                    AF.Exp, scale=0.125)
                                # diagonal masks: even -> tril, odd -> flag
                                nc.vector.tensor_tensor(
                                    ex[:, :, 0:P], ex[:, :, 0:P],
                                    mask2_m, ALU.mult)
                                for i in range(2):
                                    nc.tensor.matmul(
                                        avs[hh][:, q0:TQ],
                                        v_aug[2 * j + i][:, h, :],
                                        ex[:, i, 0:w],
                                        start=(j == 0 and i == 0),
                                        stop=(j == NQT - 1 and i == 1),
                                        skip_group_check=True)
                        for hh in range(2):
                            av = avs[hh]
                            sums_bf = att.tile([1, TQ], BF16, tag="sums")
                            nc.vector.tensor_copy(sums_bf, av[HD:HD + 1, :])
                            dn = ps_sc.tile([HD, TQ], F32, tag="sc")
                            nc.tensor.matmul(dn, ones_hd, sums_bf,
                                             start=True, stop=True)
                            rb = att.tile([HD, TQ], F32, tag="rb")
                            with nc.allow_low_precision(reason="softmax denom"):
                                nc.vector.reciprocal_approx_fast(rb, dn)
                            nc.vector.tensor_tensor(
                                yTp[m][hh * HD:(hh + 1) * HD, :],
                                av[0:HD, :], rb, ALU.mult)

            if dbg:
                for m in range(NPAIR):
                    nc.sync.dma_start(dbg_k[m * P:(m + 1) * P, :], kTp[m])
                    nc.sync.dma_start(dbg_q[m * P:(m + 1) * P, :], qTp[m])
                    nc.sync.dma_start(dbg_y[m * P:(m + 1) * P, :], yTp[m])
                for t in range(NT):
                    nc.sync.dma_start(
                        dbg_v[t * P:(t + 1) * P, :],
                        v_aug[t].rearrange("p h d -> p (h d)"))

            # ---- phase D: Wo + bias + residual -> x1T ----
            with tc.tile_pool(name="ps_wo", bufs=2, space="PSUM") as ps_wo:
                for m in range(CJ):
                    pm = ps_wo.tile([P, TQ], F32, tag="wops")
                    for kc in range(CJ):
                        nc.tensor.matmul(
                            pm, wo_t[:, kc, m * P:(m + 1) * P], yTp[kc],
                            start=(kc == 0), stop=(kc == CJ - 1))
                    nc.vector.tensor_scalar(
                        x1T[m], pm, bob_t[:, m:m + 1], None, ALU.add)
                    nc.vector.tensor_tensor(
                        x1T[m], x1T[m], xTo[m], ALU.add)

        if dbg:
            for m in range(CJ):
                nc.sync.dma_start(dbg_x1[m * P:(m + 1) * P, :], x1T[m])

        ctx_ad.close()

        # ---------------- phases E-G: LN2, FC+gelu, proj ----------------
        with tc.tile_pool(name="mlp_live", bufs=1) as mlp_live:
            h1T = [mlp_live.tile([P, TQ], BF16, tag=f"h1T{m}", name=f"h1T{m}")
                   for m in range(FCJ)]
            xh2 = [mlp_live.tile([P, TQ], BF16, tag=f"xh2{m}", name=f"xh2{m}")
                   for m in range(CJ)]

            # ---- phase E: LN2 ----
            with tc.tile_pool(name="ph_e", bufs=2) as ph_e, \
                 tc.tile_pool(name="lne", bufs=1) as lne, \
                 tc.tile_pool(name="ps_st2", bufs=1, space="PSUM") as ps_st2, \
                 tc.tile_pool(name="ps_bc2", bufs=1, space="PSUM") as ps_bc2:
                mu_ps = ps_st2.tile([1, TQ], F32, tag="mups2", name="mups2")
                sq_ps = ps_st2.tile([1, TQ], F32, tag="sqps2", name="sqps2")
                for m in range(CJ):
                    b = ph_e.tile([P, TQ], BF16, tag="x1bf")
                    s = ph_e.tile([P, TQ], BF16, tag="x1sq")
                    nc.vector.tensor_copy(b, x1T[m])
                    nc.vector.tensor_tensor(s, b, b, ALU.mult)
                    nc.tensor.matmul(mu_ps, onesc_bf, b,
                                     start=(m == 0), stop=(m == CJ - 1),
                                     skip_group_check=True)
                    nc.tensor.matmul(sq_ps, onesc_bf, s,
                                     start=(m == 0), stop=(m == CJ - 1),
                                     skip_group_check=True)
                mu_f = lne.tile([1, TQ], F32, tag="muf2")
                nc.vector.tensor_scalar(mu_f, mu_ps, 1.0 / C, None, ALU.mult)
                var_f = lne.tile([1, TQ], F32, tag="varf2")
                nc.vector.tensor_scalar(var_f, sq_ps, 1.0 / C, None, ALU.mult)
                musq = lne.tile([1, TQ], F32, tag="musq2")
                nc.vector.tensor_tensor(musq, mu_f, mu_f, ALU.mult)
                nc.vector.tensor_tensor(var_f, var_f, musq, ALU.subtract)
                lv_f = lne.tile([1, TQ], F32, tag="lvf2")
                nc.scalar.activation(lv_f, var_f, AF.Ln, bias=eps1)
                rstd_f = lne.tile([1, TQ], F32, tag="rstdf2")
                nc.scalar.activation(rstd_f, lv_f, AF.Exp, scale=-0.5)
                nmr_f = lne.tile([1, TQ], F32, tag="nmrf2")
                nc.vector.tensor_tensor(nmr_f, mu_f, rstd_f, ALU.mult)
                rstd_bf = lne.tile([1, TQ], BF16, tag="rstdbf2")
                nc.vector.tensor_copy(rstd_bf, rstd_f)
                nmr_bf = lne.tile([1, TQ], BF16, tag="nmrbf2")
                nc.vector.tensor_copy(nmr_bf, nmr_f)
                rst_ps = ps_bc2.tile([P, TQ], F32, tag="rstps2", name="rstps2")
                nrm_ps = ps_bc2.tile([P, TQ], F32, tag="nrmps2", name="nrmps2")
                nc.tensor.matmul(rst_ps, ones_row, rstd_bf,
                                 start=True, stop=True)
                nc.tensor.matmul(nrm_ps, mones_row, nmr_bf,
                                 start=True, stop=True)
                rst_s = lne.tile([P, TQ], F32, tag="rsts2")
                nc.vector.tensor_copy(rst_s, rst_ps)
                nrm_s = lne.tile([P, TQ], F32, tag="nrms2")
                nc.scalar.copy(nrm_s, nrm_ps)
                for m in range(CJ):
                    tmp = ph_e.tile([P, TQ], F32, tag="xnt2")
                    nc.vector.tensor_tensor(tmp, x1T[m], rst_s, ALU.mult)
                    nc.vector.tensor_tensor(xh2[m], tmp, nrm_s, ALU.add)

            # ---- phase F: FC + gelu ----
            with tc.tile_pool(name="wfc_p", bufs=3) as wfc_p, \
                 tc.tile_pool(name="ph_f", bufs=3) as ph_f, \
                 tc.tile_pool(name="ps_fc", bufs=4, space="PSUM") as ps_fc:
                for mo in range(FCJ):
                    if mo % 4 == 0:
                        if mo == 0:
                            wt4 = wfc0_t
                        else:
                            wt4 = wfc_p.tile([P, CJ, 4 * P], BF16, tag="wfc")
                            nc.sync.dma_start(
                                wt4, wfc_d[:, :, mo * P:(mo + 4) * P])
                    ml = mo % 4
                    pf = ps_fc.tile([P, TQ], F32, tag="fc")
                    for kc in range(CJ):
                        nc.tensor.matmul(pf, wt4[:, kc, ml * P:(ml + 1) * P],
                                         xh2[kc],
                                         start=(kc == 0), stop=(kc == CJ - 1))
                    xb = ph_f.tile([P, TQ], BF16, tag="xb")
                    nc.scalar.activation(xb, pf, AF.Identity,
                                         bias=bfcb_t[:, mo:mo + 1])
                    s1 = ph_f.tile([P, TQ], BF16, tag="s1")
                    s2 = ph_f.tile([P, TQ], BF16, tag="s2")
                    nc.vector.tensor_tensor(s1, xb, xb, ALU.mult)
                    nc.vector.tensor_tensor(s2, s1, s1, ALU.mult)
                    u = ph_f.tile([P, TQ], BF16, tag="u")
                    nc.scalar.activation(u, s2, AF.Sigmoid, scale=2.0 * GA2)
                    nc.vector.tensor_tensor(h1T[mo], xb, u, ALU.mult)

            if dbg:
                for m in range(CJ):
                    nc.sync.dma_start(dbg_xh2[m * P:(m + 1) * P, :], xh2[m])
                for m in range(FCJ):
                    nc.sync.dma_start(dbg_h1[m * P:(m + 1) * P, :], h1T[m])

            # ---- phase G: proj + bias + residual -> out (m-outer) ----
            with tc.tile_pool(name="ph_g", bufs=2) as ph_g, \
                 tc.tile_pool(name="ps_pj", bufs=2, space="PSUM") as ps_pj:
                for m in range(CJ):
                    pg = ps_pj.tile([P, TQ], F32, tag="pj")
                    for kc in range(FCJ):
                        nc.tensor.matmul(
                            pg, wpj_t[:, kc, m * P:(m + 1) * P], h1T[kc],
                            start=(kc == 0), stop=(kc == FCJ - 1))
                    oj = ph_g.tile([P, TQ], F32, tag="oj")
                    nc.vector.tensor_scalar(
                        oj, pg, bpjb_t[:, m:m + 1], None, ALU.add)
                    nc.vector.tensor_tensor(oj, oj, x1T[m], ALU.add)
                    nc.sync.dma_start(out_d[m * P:(m + 1) * P, :], oj)

    nc.compile()
    return nc


def _get_nc():
    if "nc" not in _CACHED:
        _CACHED["nc"] = _build_nc()
    return _CACHED["nc"]


def _perm_blocks(p):
    return [p, 1 - p, 2 + p, 3 - p, 4 + p, 5 - p, 6 + p, 7 - p]


def _build_in_maps(x, ln1_scale, ln1_bias, Wqkv, bqkv, Wo, bo,
                   ln2_scale, ln2_bias, Wfc, bfc, Wproj, bproj):
    bf16 = ml_dtypes.bfloat16
    x = np.asarray(x, np.float32)
    # Fold LN scale/bias into the following projection (exact):
    #   ln(x)*s + b  @ W  ==  ln(x) @ (s[:,None]*W)  +  b @ W
    Wq64 = np.asarray(ln1_scale, np.float64)[:, None] * np.asarray(Wqkv, np.float64)
    bq64 = np.asarray(bqkv, np.float64) + np.asarray(ln1_bias, np.float64) @ Wq64
    Wfc64 = np.asarray(ln2_scale, np.float64)[:, None] * np.asarray(Wfc, np.float64)
    bfc64 = np.asarray(bfc, np.float64) + np.asarray(ln2_bias, np.float64) @ Wfc64
    # reference packs qkv per head: [h0: q|k|v, h1: q|k|v, ...] -> head-major Q,K,V
    colmap = np.arange(3 * C).reshape(H, 3, HD)
    Wq64 = Wq64.astype(np.float32)
    bq64 = bq64.astype(np.float32)

    def pcm(w):  # [C, n] -> [128, CJ, n] (contraction chunk-major)
        n = w.shape[1]
        return np.ascontiguousarray(
            w.reshape(CJ, P, n).transpose(1, 0, 2).astype(bf16))

    wq_h = Wq64[:, colmap[:, 0, :].ravel()]
    wk_h = Wq64[:, colmap[:, 1, :].ravel()]
    wv_h = Wq64[:, colmap[:, 2, :].ravel()]
    bq_h = bq64[colmap[:, 0, :].ravel()]
    bk_h = bq64[colmap[:, 1, :].ravel()]
    bv_h = bq64[colmap[:, 2, :].ravel()]
    bkq = np.concatenate([bk_h.reshape(NPAIR, P).T,
                          bq_h.reshape(NPAIR, P).T], axis=1)

    shared = {
        "wv": pcm(wv_h), "wk": pcm(wk_h), "wq": pcm(wq_h),
        "wo": pcm(np.asarray(Wo, np.float32)),
        "wfc": pcm(Wfc64.astype(np.float32)),
        "wpj": np.ascontiguousarray(
            np.asarray(Wproj, np.float32).reshape(FCJ, P, C)
            .transpose(1, 0, 2).astype(bf16)),
        "bkq": np.ascontiguousarray(bkq, dtype=np.float32),
        "bv": np.ascontiguousarray(bv_h, dtype=np.float32),
        "bob": np.ascontiguousarray(
            np.asarray(bo, np.float32).reshape(CJ, P).T),
        "bfcb": np.ascontiguousarray(bfc64.astype(np.float32).reshape(FCJ, P).T),
        "bpjb": np.ascontiguousarray(
            np.asarray(bproj, np.float32).reshape(CJ, P).T),
    }
    tril = (np.arange(P)[None, :] >= np.arange(P)[:, None]).astype(np.float32)
    in_maps = []
    own_toks = []
    for c in range(N_CORES):
        s, p = divmod(c, 2)
        blocks = _perm_blocks(p)
        tok = np.concatenate([np.arange(b * P, (b + 1) * P) for b in blocks])
        own = np.concatenate([np.arange(b * P, (b + 1) * P)
                              for b in blocks[0::2]])
        own_toks.append((s, own))
        mask2 = np.concatenate(
            [tril, np.full((P, P), float(p), np.float32)], axis=1)
        xtl = np.ascontiguousarray(x[s][tok].T)
        in_maps.append({
            "xt": np.ascontiguousarray(xtl.astype(bf16)),
            "xto": np.ascontiguousarray(x[s][own].T),
            "mask2": np.ascontiguousarray(mask2.astype(bf16)),
            **shared,
        })
    return in_maps, own_toks


def kernel(x, ln1_scale, ln1_bias, Wqkv, bqkv, Wo, bo,
           ln2_scale, ln2_bias, Wfc, bfc, Wproj, bproj):
    from concourse.bass_utils import run_bass_kernel_spmd

    in_maps, own_toks = _build_in_maps(
        x, ln1_scale, ln1_bias, Wqkv, bqkv, Wo, bo,
        ln2_scale, ln2_bias, Wfc, bfc, Wproj, bproj)
    nc = _get_nc()
    res = run_bass_kernel_spmd(nc, in_maps, list(range(N_CORES)))

    out = np.empty((B, T, C), np.float32)
    for c in range(N_CORES):
        s, own = own_toks[c]
        out[s][own] = res.results[c]["out"].T
    return out
